# revision 6
# baseline (speedup 1.0000x reference)
"""Distributed Bass kernel for sliding-window GQA attention on 8 TRN2 NeuronCores.

Problem: B=2, S=2048, DIM=2048, H=16, KVH=4, HD=128, WINDOW=1024 (causal
sliding window), nonstandard RoPE producing 1.5*HD score features.

Sharding: each core owns one (batch, 512-row q-slice) pair — no collectives.
Each core loads the transposed x context rows [q0-1024, q0+512) (zero-padded
below row 0), computes Q/K/V projections + rope scaling, block-sparse
attention (9 k-blocks of 128 per 128-row q-block, uniform across cores via
local window coordinates), unnormalized exp softmax (scores are bounded;
padding keys are killed by a -1e9 bias folded into an extra score feature),
a transpose-by-matmul against diag(1/rowsum) that normalizes for free, PV,
and the output projection against the full wo. Host concatenates the 8
(512, 2048) output slices.
"""
import numpy as np
import ml_dtypes

import concourse.tile as tile
from concourse import bacc, mybir
from concourse.bass_utils import run_bass_kernel_spmd
from contextlib import ExitStack

F32 = mybir.dt.float32
BF16 = mybir.dt.bfloat16
EXP = mybir.ActivationFunctionType.Exp

B, S, DIM = 2, 2048, 2048
H, KVH, HD = 16, 4, 128
WINDOW = 1024
SCALE = HD ** -0.5
SQ = 512          # q rows per core
CTX = 1536        # context rows per core (WINDOW + SQ)
NDC = DIM // 128  # 16 dim chunks
NEG = -1e9

_cache = {}


def _build():
    nc = bacc.Bacc("TRN2", target_bir_lowering=False, debug=False, num_devices=8)

    xt_d = nc.dram_tensor("xt", [128, NDC * CTX], BF16, kind="ExternalInput")
    wq_d = nc.dram_tensor("wq", [128, 8 * NDC * 256], BF16, kind="ExternalInput")
    wk_d = nc.dram_tensor("wk", [128, 2 * 8 * 512], BF16, kind="ExternalInput")
    wv_d = nc.dram_tensor("wv", [128, 2 * 8 * 512], BF16, kind="ExternalInput")
    wo_d = nc.dram_tensor("wo", [128, 8 * 2 * 2048], BF16, kind="ExternalInput")
    fqm_d = nc.dram_tensor("fqm", [64, SQ], F32, kind="ExternalInput")
    fqp_d = nc.dram_tensor("fqp", [64, SQ], F32, kind="ExternalInput")
    fkm_d = nc.dram_tensor("fkm", [64, CTX], F32, kind="ExternalInput")
    fkp_d = nc.dram_tensor("fkp", [64, CTX], F32, kind="ExternalInput")
    t0_d = nc.dram_tensor("t0", [128, 128], F32, kind="ExternalInput")
    t8_d = nc.dram_tensor("t8", [128, 128], F32, kind="ExternalInput")
    padb_d = nc.dram_tensor("padb", [1, CTX], BF16, kind="ExternalInput")
    id_d = nc.dram_tensor("ident", [128, 128], BF16, kind="ExternalInput")
    out_d = nc.dram_tensor("out", [SQ, DIM], F32, kind="ExternalOutput")

    with tile.TileContext(nc) as tc, ExitStack() as ctx:
        xp = ctx.enter_context(tc.tile_pool(name="xp", bufs=1))
        wp = ctx.enter_context(tc.tile_pool(name="wp", bufs=3))
        cp = ctx.enter_context(tc.tile_pool(name="cp", bufs=1))
        qp = ctx.enter_context(tc.tile_pool(name="qp", bufs=1))
        kp = ctx.enter_context(tc.tile_pool(name="kp", bufs=1))
        vp = ctx.enter_context(tc.tile_pool(name="vp", bufs=1))
        pp = ctx.enter_context(tc.tile_pool(name="pp", bufs=2))
        ptp = ctx.enter_context(tc.tile_pool(name="ptp", bufs=2))
        dgp = ctx.enter_context(tc.tile_pool(name="dgp", bufs=2))
        smp = ctx.enter_context(tc.tile_pool(name="smp", bufs=8))
        ap_ = ctx.enter_context(tc.tile_pool(name="ap", bufs=1))
        op_ = ctx.enter_context(tc.tile_pool(name="op", bufs=2))
        ps = ctx.enter_context(tc.tile_pool(name="ps", bufs=5, space="PSUM"))
        sps = ctx.enter_context(tc.tile_pool(name="sps", bufs=3, space="PSUM"))

        # ---- constants ----
        fqm = cp.tile([64, SQ], F32, tag="fqm")
        nc.sync.dma_start(fqm[:], fqm_d[:, :])
        fqp = cp.tile([64, SQ], F32, tag="fqp")
        nc.sync.dma_start(fqp[:], fqp_d[:, :])
        fkm = cp.tile([64, CTX], F32, tag="fkm")
        nc.sync.dma_start(fkm[:], fkm_d[:, :])
        fkp = cp.tile([64, CTX], F32, tag="fkp")
        nc.sync.dma_start(fkp[:], fkp_d[:, :])
        t0 = cp.tile([128, 128], F32, tag="t0")
        nc.sync.dma_start(t0[:], t0_d[:, :])
        t8 = cp.tile([128, 128], F32, tag="t8")
        nc.sync.dma_start(t8[:], t8_d[:, :])
        ident = cp.tile([128, 128], BF16, tag="ident")
        nc.sync.dma_start(ident[:], id_d[:, :])

        # ---- x (transposed, bf16) : [128, dc, CTX] in 4 DMA chunks ----
        x_sb = xp.tile([128, NDC, CTX], BF16, tag="x")
        for g in range(4):
            nc.sync.dma_start(
                x_sb[:, g * 4 : (g + 1) * 4, :],
                xt_d[:, g * 4 * CTX : (g + 1) * 4 * CTX],
            )

        # ---- K/V projections ----
        # wk/wv slots: [128, 8, 512] (dc half h2)
        wk_t = [wp.tile([128, 8, 512], BF16, tag="w", name=f"wk{i}") for i in range(2)]
        for h2 in range(2):
            nc.sync.dma_start(wk_t[h2][:], wk_d[:, h2 * 4096 : (h2 + 1) * 4096])
        wv_t = [wp.tile([128, 8, 512], BF16, tag="w", name=f"wv{i}") for i in range(2)]
        for h2 in range(2):
            nc.sync.dma_start(wv_t[h2][:], wv_d[:, h2 * 4096 : (h2 + 1) * 4096])

        k1 = kp.tile([128, KVH, CTX], BF16, tag="k1")
        k2 = kp.tile([65, KVH, CTX], BF16, tag="k2")
        for kv in range(KVH):
            nc.sync.dma_start(k2[64:65, kv, :], padb_d[:, :])
        v_sb = vp.tile([128, 12, 512], BF16, tag="v")

        for kv in range(KVH):
            for ck in range(3):  # k columns, 512 each
                kps = ps.tile([128, 512], F32, tag="ps")
                for dc in range(NDC):
                    nc.tensor.matmul(
                        kps[:],
                        wk_t[dc // 8][:, dc % 8, kv * 128 : (kv + 1) * 128],
                        x_sb[:, dc, ck * 512 : (ck + 1) * 512],
                        start=(dc == 0),
                        stop=(dc == NDC - 1),
                    )
                nc.vector.tensor_mul(
                    k1[0:64, kv, ck * 512 : (ck + 1) * 512],
                    kps[0:64, :],
                    fkm[:, ck * 512 : (ck + 1) * 512],
                )
                nc.vector.tensor_mul(
                    k1[64:128, kv, ck * 512 : (ck + 1) * 512],
                    kps[0:64, :],
                    fkp[:, ck * 512 : (ck + 1) * 512],
                )
                nc.scalar.copy(k2[0:64, kv, ck * 512 : (ck + 1) * 512], kps[64:128, :])

        for kb in range(12):
            vps = ps.tile([128, 512], F32, tag="ps")
            for dc in range(NDC):
                nc.tensor.matmul(
                    vps[:],
                    x_sb[:, dc, kb * 128 : (kb + 1) * 128],
                    wv_t[dc // 8][:, dc % 8, :],
                    start=(dc == 0),
                    stop=(dc == NDC - 1),
                )
            nc.any.tensor_copy(v_sb[:, kb, :], vps[:])

        # ---- Q projection ----
        q1 = qp.tile([128, H, SQ], BF16, tag="q1")
        q2 = qp.tile([65, H, SQ], BF16, tag="q2")
        nc.vector.memset(q2[64:65, :, :], 1.0)
        for hg in range(8):  # head pairs
            wq_t = wp.tile([128, NDC, 256], BF16, tag="w")
            nc.sync.dma_start(wq_t[:], wq_d[:, hg * NDC * 256 : (hg + 1) * NDC * 256])
            for hh in range(2):
                h = hg * 2 + hh
                qps = ps.tile([128, 512], F32, tag="ps")
                for dc in range(NDC):
                    nc.tensor.matmul(
                        qps[:],
                        wq_t[:, dc, hh * 128 : (hh + 1) * 128],
                        x_sb[:, dc, 1024:1536],
                        start=(dc == 0),
                        stop=(dc == NDC - 1),
                    )
                nc.vector.tensor_mul(q1[0:64, h, :], qps[0:64, :], fqm[:])
                nc.vector.tensor_mul(q1[64:128, h, :], qps[0:64, :], fqp[:])
                nc.scalar.copy(q2[0:64, h, :], qps[64:128, :])

        # ---- attention ----
        attn = ap_.tile([128, H, SQ], BF16, tag="attn")
        for h in range(H):
            kv = h // 4
            aps = ps.tile([128, 512], F32, tag="ps")  # attn^T psum, held per head
            for qc in range(4):
                qb = qc * 128
                # scores S[q=128, k=1152] in 3 psum chunks of 384
                schunks = []
                for ckn in range(3):
                    sp = sps.tile([128, 384], F32, tag="s")
                    lo = qb + ckn * 384
                    nc.tensor.matmul(
                        sp[:],
                        q1[:, h, qb : qb + 128],
                        k1[:, kv, lo : lo + 384],
                        start=True,
                        stop=False,
                    )
                    nc.tensor.matmul(
                        sp[:],
                        q2[:, h, qb : qb + 128],
                        k2[:, kv, lo : lo + 384],
                        start=False,
                        stop=True,
                    )
                    schunks.append(sp)
                # masks: window-tail triangle on k-block 0, causal triangle on k-block 8
                nc.vector.tensor_add(schunks[0][:, 0:128], schunks[0][:, 0:128], t0[:])
                nc.vector.tensor_add(
                    schunks[2][:, 256:384], schunks[2][:, 256:384], t8[:]
                )
                # exp + row sums
                p_sb = pp.tile([128, 1152], BF16, tag="p")
                acc = smp.tile([128, 3], F32, tag="acc")
                for ckn in range(3):
                    nc.scalar.activation(
                        p_sb[:, ckn * 384 : (ckn + 1) * 384],
                        schunks[ckn][:],
                        EXP,
                        accum_out=acc[:, ckn : ckn + 1],
                    )
                sm = smp.tile([128, 1], F32, tag="sm")
                nc.vector.tensor_add(sm[:], acc[:, 0:1], acc[:, 1:2])
                nc.vector.tensor_add(sm[:], sm[:], acc[:, 2:3])
                rc = smp.tile([128, 1], F32, tag="rc")
                nc.vector.reciprocal(rc[:], sm[:])
                dg = dgp.tile([128, 128], BF16, tag="dg")
                nc.vector.tensor_scalar_mul(dg[:], ident[:], rc[:, 0:1])
                # normalized transpose: PT[k,q] = P^T @ diag(1/sum), 9 blocks
                pt_sb = ptp.tile([128, 1152], BF16, tag="pt")
                for ckn in range(3):
                    ptps = ps.tile([128, 512], F32, tag="ps")
                    for t in range(3):
                        m = ckn * 3 + t
                        nc.tensor.matmul(
                            ptps[:, t * 128 : (t + 1) * 128],
                            p_sb[:, m * 128 : (m + 1) * 128],
                            dg[:],
                            start=True,
                            stop=True,
                        )
                    nc.any.tensor_copy(
                        pt_sb[:, ckn * 384 : (ckn + 1) * 384], ptps[:, 0:384]
                    )
                # PV: attn^T[hd, q] += V[kb]^T-contract over k blocks
                for m in range(9):
                    nc.tensor.matmul(
                        aps[:, qb : qb + 128],
                        v_sb[:, qc + m, kv * 128 : (kv + 1) * 128],
                        pt_sb[:, m * 128 : (m + 1) * 128],
                        start=(m == 0),
                        stop=(m == 8),
                    )
            nc.any.tensor_copy(attn[:, h, :], aps[:])

        # ---- output projection (wo streamed per output-column block dn) ----
        for dn in range(4):
            wo_t = [wp.tile([128, 8, 512], BF16, tag="w", name=f"wo{dn}_{i}") for i in range(2)]
            for g in range(2):
                nc.sync.dma_start(
                    wo_t[g][:], wo_d[:, (dn * 2 + g) * 4096 : (dn * 2 + g + 1) * 4096]
                )
            for qc in range(4):
                ops = ps.tile([128, 512], F32, tag="ps")
                for f in range(H):
                    nc.tensor.matmul(
                        ops[:],
                        attn[:, f, qc * 128 : (qc + 1) * 128],
                        wo_t[f // 8][:, f % 8, :],
                        start=(f == 0),
                        stop=(f == H - 1),
                    )
                o_sb = op_.tile([128, 512], F32, tag="o")
                nc.any.tensor_copy(o_sb[:], ops[:])
                nc.sync.dma_start(
                    out_d[qc * 128 : (qc + 1) * 128, dn * 512 : (dn + 1) * 512],
                    o_sb[:],
                )

    nc.compile()
    return nc


def _prep_core(inputs, c):
    x = inputs["x"]
    cos, sin = np.asarray(inputs["cos"]), np.asarray(inputs["sin"])
    mask = np.asarray(inputs["mask"])
    wq = np.asarray(inputs["wq"], dtype=np.float32)
    wk = np.asarray(inputs["wk"], dtype=np.float32)
    wv = np.asarray(inputs["wv"], dtype=np.float32)
    wo = np.asarray(inputs["wo"], dtype=np.float32)
    bf = ml_dtypes.bfloat16
    b, qs = c // 4, c % 4
    q0 = qs * SQ
    lo = max(0, WINDOW - q0)  # first valid context col

    # x transposed [dim, ctx] -> [128, dc, CTX]
    xt = np.zeros((DIM, CTX), dtype=np.float32)
    xt[:, lo:] = np.asarray(x[b], dtype=np.float32)[q0 - WINDOW + lo : q0 + SQ, :].T
    xt = np.ascontiguousarray(
        xt.reshape(NDC, 128, CTX).transpose(1, 0, 2).reshape(128, NDC * CTX)
    ).astype(bf)

    # weights (SCALE folded into wq) in sbuf-friendly layouts
    wqh = (wq * SCALE).reshape(NDC, 128, 8, 256).transpose(1, 2, 0, 3)
    wqh = np.ascontiguousarray(wqh).reshape(128, 8 * NDC * 256).astype(bf)
    wkh = wk.reshape(2, 8, 128, 512).transpose(2, 0, 1, 3)
    wkh = np.ascontiguousarray(wkh).reshape(128, 2 * 8 * 512).astype(bf)
    wvh = wv.reshape(2, 8, 128, 512).transpose(2, 0, 1, 3)
    wvh = np.ascontiguousarray(wvh).reshape(128, 2 * 8 * 512).astype(bf)
    woh = wo.reshape(2, 8, 128, 4, 512).transpose(2, 3, 0, 1, 4)
    woh = np.ascontiguousarray(woh).reshape(128, 8 * 2 * 2048).astype(bf)

    # rope factor tables, transposed
    fqm = np.ascontiguousarray((cos - sin)[q0 : q0 + SQ].T, dtype=np.float32)
    fqp = np.ascontiguousarray((cos + sin)[q0 : q0 + SQ].T, dtype=np.float32)
    fkm = np.zeros((64, CTX), dtype=np.float32)
    fkp = np.zeros((64, CTX), dtype=np.float32)
    fkm[:, lo:] = (cos - sin)[q0 - WINDOW + lo : q0 + SQ].T
    fkp[:, lo:] = (cos + sin)[q0 - WINDOW + lo : q0 + SQ].T

    # mask tiles (constant across 128-row q-blocks; sliced from the real mask)
    def mslice(r0, c0):
        t = np.full((128, 128), NEG, dtype=np.float32)
        if c0 >= 0:
            t[:, :] = mask[r0 : r0 + 128, c0 : c0 + 128]
        return t

    t0 = mslice(q0, q0 - WINDOW)
    t8 = mslice(q0, q0)
    padb = np.zeros((1, CTX), dtype=np.float32)
    padb[0, :lo] = NEG
    return {
        "xt": xt, "wq": wqh, "wk": wkh, "wv": wvh, "wo": woh,
        "fqm": fqm, "fqp": fqp, "fkm": fkm, "fkp": fkp,
        "t0": t0, "t8": t8, "padb": padb.astype(bf),
        "ident": np.eye(128, dtype=np.float32).astype(bf),
    }


def kernel(**inputs) -> np.ndarray:
    if "nc" not in _cache:
        _cache["nc"] = _build()
    nc = _cache["nc"]
    in_maps = [_prep_core(inputs, c) for c in range(8)]
    res = run_bass_kernel_spmd(nc, in_maps, core_ids=list(range(8)))
    out = np.zeros((B, S, DIM), dtype=np.float32)
    for c in range(8):
        b, qs = c // 4, c % 4
        out[b, qs * SQ : (qs + 1) * SQ, :] = res.results[c]["out"]
    return out


# revision 7
# speedup vs baseline: 1.1502x; 1.1502x over previous
"""Distributed Bass kernel for sliding-window GQA attention on 8 TRN2 NeuronCores.

Problem: B=2, S=2048, DIM=2048, H=16, KVH=4, HD=128, WINDOW=1024 (causal
sliding window), nonstandard RoPE producing 1.5*HD score features.

Sharding (tensor-parallel on the kv-head axis, data-parallel on batch —
no collectives): core c owns (batch, kv-group) = (c//4, c%4): its 4 q-heads
and 1 kv head over the full 2048-row sequence. wq/wk/wv are column-sharded
by kv group, wo row-sharded. Each core emits a PARTIAL output projection
(its 4 heads x its wo rows); the host sums the 4 partials per batch while
unsharding — replacing the all-reduce.

Per core: Q/K/V projections + rope scaling, block-sparse sliding-window
attention in global coordinates (k-blocks max(0,qc-8)..qc per 128-row
q-block qc), unnormalized exp softmax (bounded scores, no max pass), a
transpose-by-matmul against diag(1/rowsum) that normalizes for free, PV,
and the partial O-projection.
"""
import numpy as np
import ml_dtypes

import concourse.tile as tile
from concourse import bacc, mybir
from concourse.bass_utils import run_bass_kernel_spmd
from contextlib import ExitStack

F32 = mybir.dt.float32
BF16 = mybir.dt.bfloat16
EXP = mybir.ActivationFunctionType.Exp

B, S, DIM = 2, 2048, 2048
H, KVH, HD = 16, 4, 128
HPC = H // KVH  # heads per core (4)
WINDOW = 1024
SCALE = HD ** -0.5
NDC = DIM // 128  # 16 dim chunks
NQC = S // 128    # 16 q blocks

_cache = {}


def _kblocks(qc):
    return list(range(max(0, qc - 8), qc + 1))


def _build():
    nc = bacc.Bacc("TRN2", target_bir_lowering=False, debug=False, num_devices=8)

    xt_d = nc.dram_tensor("xt", [128, 4 * NDC * 512], BF16, kind="ExternalInput")
    wq_d = nc.dram_tensor("wq", [128, 2 * NDC * 256], BF16, kind="ExternalInput")
    wkv_d = nc.dram_tensor("wkv", [128, NDC * 256], BF16, kind="ExternalInput")
    wo_d = nc.dram_tensor("wo", [128, 2 * 2 * 2048], BF16, kind="ExternalInput")
    fm_d = nc.dram_tensor("fm", [64, S], F32, kind="ExternalInput")
    fp_d = nc.dram_tensor("fp", [64, S], F32, kind="ExternalInput")
    t0_d = nc.dram_tensor("t0", [128, 128], F32, kind="ExternalInput")
    t8_d = nc.dram_tensor("t8", [128, 128], F32, kind="ExternalInput")
    id_d = nc.dram_tensor("ident", [128, 128], BF16, kind="ExternalInput")
    out_d = nc.dram_tensor("out", [S, DIM], F32, kind="ExternalOutput")

    with tile.TileContext(nc) as tc, ExitStack() as ctx:
        xp = ctx.enter_context(tc.tile_pool(name="xp", bufs=3))
        wp = ctx.enter_context(tc.tile_pool(name="wp", bufs=3))
        cp = ctx.enter_context(tc.tile_pool(name="cp", bufs=1))
        qp = ctx.enter_context(tc.tile_pool(name="qp", bufs=1))
        kp = ctx.enter_context(tc.tile_pool(name="kp", bufs=1))
        vp = ctx.enter_context(tc.tile_pool(name="vp", bufs=1))
        pp = ctx.enter_context(tc.tile_pool(name="pp", bufs=2))
        ptp = ctx.enter_context(tc.tile_pool(name="ptp", bufs=2))
        dgp = ctx.enter_context(tc.tile_pool(name="dgp", bufs=2))
        smp = ctx.enter_context(tc.tile_pool(name="smp", bufs=8))
        ap_ = ctx.enter_context(tc.tile_pool(name="ap", bufs=1))
        op_ = ctx.enter_context(tc.tile_pool(name="op", bufs=3))
        ps = ctx.enter_context(tc.tile_pool(name="ps", bufs=5, space="PSUM"))
        sps = ctx.enter_context(tc.tile_pool(name="sps", bufs=3, space="PSUM"))

        # ---- weights for phase 1 first (prologue-critical DMA order) ----
        wkv_t = wp.tile([128, NDC, 256], BF16, tag="w")  # cols: [wk 128 | wv 128]
        nc.sync.dma_start(wkv_t[:], wkv_d[:, :])
        wq_t = [
            wp.tile([128, NDC, 256], BF16, tag="w", name=f"wq{i}") for i in range(2)
        ]
        for i in range(2):
            nc.sync.dma_start(wq_t[i][:], wq_d[:, i * NDC * 256 : (i + 1) * NDC * 256])

        # ---- constants ----
        fm = cp.tile([64, S], F32, tag="fm")
        nc.sync.dma_start(fm[:], fm_d[:, :])
        fp = cp.tile([64, S], F32, tag="fp")
        nc.sync.dma_start(fp[:], fp_d[:, :])
        t0 = cp.tile([128, 128], F32, tag="t0")
        nc.sync.dma_start(t0[:], t0_d[:, :])
        t8 = cp.tile([128, 128], F32, tag="t8")
        nc.sync.dma_start(t8[:], t8_d[:, :])
        ident = cp.tile([128, 128], BF16, tag="ident")
        nc.sync.dma_start(ident[:], id_d[:, :])

        q1 = qp.tile([128, HPC, S], BF16, tag="q1")
        q2 = qp.tile([64, HPC, S], BF16, tag="q2")
        k1 = kp.tile([128, S], BF16, tag="k1")
        k2 = kp.tile([64, S], BF16, tag="k2")
        v_sb = vp.tile([128, NQC, 128], BF16, tag="v")

        # ---- projections, streamed per x column-quarter ----
        for cq in range(4):
            x_q = xp.tile([128, NDC, 512], BF16, tag="x")
            nc.sync.dma_start(x_q[:], xt_d[:, cq * NDC * 512 : (cq + 1) * NDC * 512])
            cs = slice(cq * 512, (cq + 1) * 512)
            fmc, fpc = fm[:, cs], fp[:, cs]

            kps = ps.tile([128, 512], F32, tag="ps")
            for dc in range(NDC):
                nc.tensor.matmul(
                    kps[:],
                    wkv_t[:, dc, 0:128],
                    x_q[:, dc, :],
                    start=(dc == 0),
                    stop=(dc == NDC - 1),
                )
            nc.vector.tensor_mul(k1[0:64, cs], kps[0:64, :], fmc)
            nc.vector.tensor_mul(k1[64:128, cs], kps[0:64, :], fpc)
            nc.scalar.copy(k2[:, cs], kps[64:128, :])

            vps = ps.tile([128, 512], F32, tag="ps")
            for kb4 in range(4):
                kb = cq * 4 + kb4
                for dc in range(NDC):
                    nc.tensor.matmul(
                        vps[:, kb4 * 128 : (kb4 + 1) * 128],
                        x_q[:, dc, kb4 * 128 : (kb4 + 1) * 128],
                        wkv_t[:, dc, 128:256],
                        start=(dc == 0),
                        stop=(dc == NDC - 1),
                    )
            for kb4 in range(4):
                nc.any.tensor_copy(
                    v_sb[:, cq * 4 + kb4, :], vps[:, kb4 * 128 : (kb4 + 1) * 128]
                )

            for h in range(HPC):
                qps = ps.tile([128, 512], F32, tag="ps")
                for dc in range(NDC):
                    nc.tensor.matmul(
                        qps[:],
                        wq_t[h // 2][:, dc, (h % 2) * 128 : (h % 2 + 1) * 128],
                        x_q[:, dc, :],
                        start=(dc == 0),
                        stop=(dc == NDC - 1),
                    )
                nc.vector.tensor_mul(q1[0:64, h, cs], qps[0:64, :], fmc)
                nc.vector.tensor_mul(q1[64:128, h, cs], qps[0:64, :], fpc)
                nc.scalar.copy(q2[:, h, cs], qps[64:128, :])

        # ---- attention ----
        attn = ap_.tile([128, HPC, S], BF16, tag="attn")
        for h in range(HPC):
            for qg in range(4):  # q groups of 512 (4 q-blocks) per attn psum
                aps = ps.tile([128, 512], F32, tag="ps")
                for qc4 in range(4):
                    qc = qg * 4 + qc4
                    qb = qc * 128
                    kbs = _kblocks(qc)
                    nkb = len(kbs)
                    chunks = [kbs[i : i + 3] for i in range(0, nkb, 3)]
                    schunks = []
                    for chunk in chunks:
                        w = len(chunk) * 128
                        sp = sps.tile([128, 384], F32, tag="s")
                        lo = chunk[0] * 128
                        nc.tensor.matmul(
                            sp[:, 0:w],
                            q1[:, h, qb : qb + 128],
                            k1[:, lo : lo + w],
                            start=True,
                            stop=False,
                        )
                        nc.tensor.matmul(
                            sp[:, 0:w],
                            q2[:, h, qb : qb + 128],
                            k2[:, lo : lo + w],
                            start=False,
                            stop=True,
                        )
                        schunks.append(sp)
                    # masks: window-tail triangle on k-block qc-8, causal on qc
                    if kbs[0] == qc - 8:
                        nc.vector.tensor_add(
                            schunks[0][:, 0:128], schunks[0][:, 0:128], t0[:]
                        )
                    dpos = (nkb - 1) % 3
                    nc.vector.tensor_add(
                        schunks[-1][:, dpos * 128 : (dpos + 1) * 128],
                        schunks[-1][:, dpos * 128 : (dpos + 1) * 128],
                        t8[:],
                    )
                    # exp + row sums
                    p_sb = pp.tile([128, 1152], BF16, tag="p")
                    acc = smp.tile([128, 3], F32, tag="acc")
                    for ci, chunk in enumerate(chunks):
                        w = len(chunk) * 128
                        nc.scalar.activation(
                            p_sb[:, ci * 384 : ci * 384 + w],
                            schunks[ci][:, 0:w],
                            EXP,
                            accum_out=acc[:, ci : ci + 1],
                        )
                    sm = smp.tile([128, 1], F32, tag="sm")
                    if len(chunks) == 1:
                        nc.vector.tensor_copy(sm[:], acc[:, 0:1])
                    else:
                        nc.vector.tensor_add(sm[:], acc[:, 0:1], acc[:, 1:2])
                        if len(chunks) == 3:
                            nc.vector.tensor_add(sm[:], sm[:], acc[:, 2:3])
                    rc = smp.tile([128, 1], F32, tag="rc")
                    nc.vector.reciprocal(rc[:], sm[:])
                    dg = dgp.tile([128, 128], BF16, tag="dg")
                    nc.vector.tensor_scalar_mul(dg[:], ident[:], rc[:, 0:1])
                    # normalized transpose: PT[k,q] = P^T @ diag(1/sum)
                    pt_sb = ptp.tile([128, 1152], BF16, tag="pt")
                    for ci, chunk in enumerate(chunks):
                        w = len(chunk) * 128
                        ptps = ps.tile([128, 512], F32, tag="ps")
                        for t in range(len(chunk)):
                            nc.tensor.matmul(
                                ptps[:, t * 128 : (t + 1) * 128],
                                p_sb[:, ci * 384 + t * 128 : ci * 384 + (t + 1) * 128],
                                dg[:],
                                start=True,
                                stop=True,
                            )
                        nc.any.tensor_copy(
                            pt_sb[:, ci * 384 : ci * 384 + w], ptps[:, 0:w]
                        )
                    # PV
                    for mi, kb in enumerate(kbs):
                        ci, t = mi // 3, mi % 3
                        nc.tensor.matmul(
                            aps[:, qc4 * 128 : (qc4 + 1) * 128],
                            v_sb[:, kb, :],
                            pt_sb[:, ci * 384 + t * 128 : ci * 384 + (t + 1) * 128],
                            start=(mi == 0),
                            stop=(mi == nkb - 1),
                        )
                nc.any.tensor_copy(attn[:, h, qg * 512 : (qg + 1) * 512], aps[:])

        # ---- partial output projection (this core's 4 heads x its wo rows) ----
        wo_t = [
            wp.tile([128, 2, 2048], BF16, tag="w", name=f"wo{i}") for i in range(2)
        ]
        for i in range(2):
            nc.sync.dma_start(wo_t[i][:], wo_d[:, i * 4096 : (i + 1) * 4096])
        for qc in range(NQC):
            for dn in range(4):
                ops = ps.tile([128, 512], F32, tag="ps")
                for f in range(HPC):
                    nc.tensor.matmul(
                        ops[:],
                        attn[:, f, qc * 128 : (qc + 1) * 128],
                        wo_t[f // 2][:, f % 2, dn * 512 : (dn + 1) * 512],
                        start=(f == 0),
                        stop=(f == HPC - 1),
                    )
                o_sb = op_.tile([128, 512], F32, tag="o")
                nc.any.tensor_copy(o_sb[:], ops[:])
                nc.sync.dma_start(
                    out_d[qc * 128 : (qc + 1) * 128, dn * 512 : (dn + 1) * 512],
                    o_sb[:],
                )

    nc.compile()
    return nc


def _prep_core(inputs, c):
    x = inputs["x"]
    cos, sin = np.asarray(inputs["cos"]), np.asarray(inputs["sin"])
    mask = np.asarray(inputs["mask"])
    wq = np.asarray(inputs["wq"], dtype=np.float32)
    wk = np.asarray(inputs["wk"], dtype=np.float32)
    wv = np.asarray(inputs["wv"], dtype=np.float32)
    wo = np.asarray(inputs["wo"], dtype=np.float32)
    bf = ml_dtypes.bfloat16
    b, g = c // 4, c % 4

    # x[b] transposed -> [128p, cq, dc, 512]
    xt = np.asarray(x[b], dtype=np.float32).T  # [dim, S]
    xt = xt.reshape(NDC, 128, 4, 512).transpose(1, 2, 0, 3)
    xt = np.ascontiguousarray(xt).reshape(128, 4 * NDC * 512).astype(bf)

    # wq slice for heads 4g..4g+3 (SCALE folded), [p, hpair, dc, 256]
    wqs = (wq[:, g * 512 : (g + 1) * 512] * SCALE).reshape(NDC, 128, 2, 256)
    wqs = np.ascontiguousarray(wqs.transpose(1, 2, 0, 3)).reshape(128, 2 * NDC * 256)
    # wk|wv slice for kv head g: [p, dc, 256] with cols [wk 128 | wv 128]
    wkv = np.concatenate(
        [wk[:, g * 128 : (g + 1) * 128], wv[:, g * 128 : (g + 1) * 128]], axis=1
    )
    wkv = np.ascontiguousarray(wkv.reshape(NDC, 128, 256).transpose(1, 0, 2)).reshape(
        128, NDC * 256
    )
    # wo rows for this core's heads: [p, f2(2 within pair), ...] tiles [128,2,2048]
    wos = wo[g * 512 : (g + 1) * 512].reshape(2, 2, 128, 2048).transpose(2, 0, 1, 3)
    wos = np.ascontiguousarray(wos).reshape(128, 2 * 2 * 2048)

    fm = np.ascontiguousarray((cos - sin).T, dtype=np.float32)
    fp_ = np.ascontiguousarray((cos + sin).T, dtype=np.float32)
    t0 = np.ascontiguousarray(mask[WINDOW : WINDOW + 128, 0:128], dtype=np.float32)
    t8 = np.ascontiguousarray(mask[0:128, 0:128], dtype=np.float32)

    return {
        "xt": xt, "wq": wqs.astype(bf), "wkv": wkv.astype(bf), "wo": wos.astype(bf),
        "fm": fm, "fp": fp_, "t0": t0, "t8": t8,
        "ident": np.eye(128, dtype=np.float32).astype(bf),
    }


def kernel(**inputs) -> np.ndarray:
    if "nc" not in _cache:
        _cache["nc"] = _build()
    nc = _cache["nc"]
    in_maps = [_prep_core(inputs, c) for c in range(8)]
    res = run_bass_kernel_spmd(nc, in_maps, core_ids=list(range(8)))
    out = np.zeros((B, S, DIM), dtype=np.float32)
    for c in range(8):
        out[c // 4] += res.results[c]["out"]
    return out


# revision 11
# speedup vs baseline: 1.1686x; 1.0160x over previous
"""Distributed Bass kernel for sliding-window GQA attention on 8 TRN2 NeuronCores.

Problem: B=2, S=2048, DIM=2048, H=16, KVH=4, HD=128, WINDOW=1024 (causal
sliding window), nonstandard RoPE producing 1.5*HD score features.

Sharding (tensor-parallel on the kv-head axis, data-parallel on batch —
no collectives): core c owns (batch, kv-group) = (c//4, c%4): its 4 q-heads
and 1 kv head over the full 2048-row sequence. wq/wk/wv are column-sharded
by kv group, wo row-sharded. Each core emits a PARTIAL output projection
(its 4 heads x its wo rows); the host sums the 4 partials per batch while
unsharding — replacing the all-reduce.

Per core: Q/K/V projections + rope scaling, block-sparse sliding-window
attention in global coordinates (k-blocks max(0,qc-8)..qc per 128-row
q-block qc), unnormalized exp softmax (bounded scores, no max pass), a
transpose-by-matmul against diag(1/rowsum) that normalizes for free, PV,
and the partial O-projection.
"""
import numpy as np
import ml_dtypes

import concourse.tile as tile
from concourse import bacc, mybir
from concourse.bass_utils import run_bass_kernel_spmd
from contextlib import ExitStack

F32 = mybir.dt.float32
BF16 = mybir.dt.bfloat16
EXP = mybir.ActivationFunctionType.Exp

B, S, DIM = 2, 2048, 2048
H, KVH, HD = 16, 4, 128
HPC = H // KVH  # heads per core (4)
WINDOW = 1024
SCALE = HD ** -0.5
NDC = DIM // 128  # 16 dim chunks
NQC = S // 128    # 16 q blocks

_cache = {}


def _kblocks(qc):
    return list(range(max(0, qc - 8), qc + 1))


def _build():
    nc = bacc.Bacc("TRN2", target_bir_lowering=False, debug=False, num_devices=8)

    xt_d = nc.dram_tensor("xt", [128, 4 * NDC * 512], BF16, kind="ExternalInput")
    wq_d = nc.dram_tensor("wq", [128, 2 * NDC * 256], BF16, kind="ExternalInput")
    wkv_d = nc.dram_tensor("wkv", [128, NDC * 256], BF16, kind="ExternalInput")
    wo_d = nc.dram_tensor("wo", [128, 2 * 2 * 2048], BF16, kind="ExternalInput")
    fm_d = nc.dram_tensor("fm", [64, S], F32, kind="ExternalInput")
    fp_d = nc.dram_tensor("fp", [64, S], F32, kind="ExternalInput")
    t0_d = nc.dram_tensor("t0", [128, 128], F32, kind="ExternalInput")
    t8_d = nc.dram_tensor("t8", [128, 128], F32, kind="ExternalInput")
    id_d = nc.dram_tensor("ident", [128, 128], BF16, kind="ExternalInput")
    out_d = nc.dram_tensor("out", [S, DIM], F32, kind="ExternalOutput")

    with tile.TileContext(nc) as tc, ExitStack() as ctx:
        xp = ctx.enter_context(tc.tile_pool(name="xp", bufs=3))
        wp = ctx.enter_context(tc.tile_pool(name="wp", bufs=3))
        cp = ctx.enter_context(tc.tile_pool(name="cp", bufs=1))
        qp = ctx.enter_context(tc.tile_pool(name="qp", bufs=1))
        kp = ctx.enter_context(tc.tile_pool(name="kp", bufs=1))
        vp = ctx.enter_context(tc.tile_pool(name="vp", bufs=1))
        pp = ctx.enter_context(tc.tile_pool(name="pp", bufs=2))
        ptp = ctx.enter_context(tc.tile_pool(name="ptp", bufs=2))
        dgp = ctx.enter_context(tc.tile_pool(name="dgp", bufs=2))
        smp = ctx.enter_context(tc.tile_pool(name="smp", bufs=8))
        ap_ = ctx.enter_context(tc.tile_pool(name="ap", bufs=1))
        op_ = ctx.enter_context(tc.tile_pool(name="op", bufs=3))
        ps = ctx.enter_context(tc.tile_pool(name="ps", bufs=5, space="PSUM"))
        sps = ctx.enter_context(tc.tile_pool(name="sps", bufs=3, space="PSUM"))

        # ---- weights for phase 1 first (prologue-critical DMA order) ----
        wkv_t = wp.tile([128, NDC, 256], BF16, tag="w")  # cols: [wk 128 | wv 128]
        nc.sync.dma_start(wkv_t[:], wkv_d[:, :])
        wq_t = [
            wp.tile([128, NDC, 256], BF16, tag="w", name=f"wq{i}") for i in range(2)
        ]
        for i in range(2):
            nc.sync.dma_start(wq_t[i][:], wq_d[:, i * NDC * 256 : (i + 1) * NDC * 256])

        q1 = qp.tile([128, HPC, S], BF16, tag="q1")
        q2 = qp.tile([64, HPC, S], BF16, tag="q2")
        k1 = kp.tile([128, S], BF16, tag="k1")
        k2 = kp.tile([64, S], BF16, tag="k2")
        v_sb = vp.tile([128, NQC, 128], BF16, tag="v")

        # ---- projections, streamed per x column-quarter ----
        fm = fp = t0 = t8 = ident = None
        for cq in range(4):
            x_q = xp.tile([128, NDC, 512], BF16, tag="x")
            for dg in range(4):
                nc.sync.dma_start(
                    x_q[:, dg * 4 : (dg + 1) * 4, :],
                    xt_d[
                        :,
                        cq * NDC * 512 + dg * 4 * 512 : cq * NDC * 512
                        + (dg + 1) * 4 * 512,
                    ],
                )
            if cq == 0:
                # constants ride after the first x chunk (not prologue-critical)
                fm = cp.tile([64, S], F32, tag="fm")
                nc.sync.dma_start(fm[:], fm_d[:, :])
                fp = cp.tile([64, S], F32, tag="fp")
                nc.sync.dma_start(fp[:], fp_d[:, :])
                t0 = cp.tile([128, 128], F32, tag="t0")
                nc.sync.dma_start(t0[:], t0_d[:, :])
                t8 = cp.tile([128, 128], F32, tag="t8")
                nc.sync.dma_start(t8[:], t8_d[:, :])
                ident = cp.tile([128, 128], BF16, tag="ident")
                nc.sync.dma_start(ident[:], id_d[:, :])
            cs = slice(cq * 512, (cq + 1) * 512)
            fmc, fpc = fm[:, cs], fp[:, cs]

            kps = ps.tile([128, 512], F32, tag="ps")
            for dc in range(NDC):
                nc.tensor.matmul(
                    kps[:],
                    wkv_t[:, dc, 0:128],
                    x_q[:, dc, :],
                    start=(dc == 0),
                    stop=(dc == NDC - 1),
                )
            nc.vector.tensor_mul(k1[0:64, cs], kps[0:64, :], fmc)
            nc.vector.tensor_mul(k1[64:128, cs], kps[0:64, :], fpc)
            nc.scalar.copy(k2[:, cs], kps[64:128, :])

            vps = ps.tile([128, 512], F32, tag="ps")
            for kb4 in range(4):
                kb = cq * 4 + kb4
                for dc in range(NDC):
                    nc.tensor.matmul(
                        vps[:, kb4 * 128 : (kb4 + 1) * 128],
                        x_q[:, dc, kb4 * 128 : (kb4 + 1) * 128],
                        wkv_t[:, dc, 128:256],
                        start=(dc == 0),
                        stop=(dc == NDC - 1),
                    )
            for kb4 in range(4):
                nc.any.tensor_copy(
                    v_sb[:, cq * 4 + kb4, :], vps[:, kb4 * 128 : (kb4 + 1) * 128]
                )

            for h in range(HPC):
                qps = ps.tile([128, 512], F32, tag="ps")
                for dc in range(NDC):
                    nc.tensor.matmul(
                        qps[:],
                        wq_t[h // 2][:, dc, (h % 2) * 128 : (h % 2 + 1) * 128],
                        x_q[:, dc, :],
                        start=(dc == 0),
                        stop=(dc == NDC - 1),
                    )
                nc.vector.tensor_mul(q1[0:64, h, cs], qps[0:64, :], fmc)
                nc.vector.tensor_mul(q1[64:128, h, cs], qps[0:64, :], fpc)
                nc.scalar.copy(q2[:, h, cs], qps[64:128, :])

        # ---- attention + interleaved partial O-projection per q-group ----
        wo_t = [
            wp.tile([128, 2, 2048], BF16, tag="w", name=f"wo{i}") for i in range(2)
        ]
        for i in range(2):
            nc.sync.dma_start(wo_t[i][:], wo_d[:, i * 4096 : (i + 1) * 4096])
        attn = ap_.tile([128, HPC, S], BF16, tag="attn")
        for qg in range(4):  # q groups of 512 (4 q-blocks) per attn psum
            for h in range(HPC):
                aps = ps.tile([128, 512], F32, tag="ps")
                for qc4 in range(4):
                    qc = qg * 4 + qc4
                    qb = qc * 128
                    kbs = _kblocks(qc)
                    nkb = len(kbs)
                    chunks = [kbs[i : i + 3] for i in range(0, nkb, 3)]
                    schunks = []
                    for chunk in chunks:
                        w = len(chunk) * 128
                        sp = sps.tile([128, 384], F32, tag="s")
                        lo = chunk[0] * 128
                        nc.tensor.matmul(
                            sp[:, 0:w],
                            q1[:, h, qb : qb + 128],
                            k1[:, lo : lo + w],
                            start=True,
                            stop=False,
                        )
                        nc.tensor.matmul(
                            sp[:, 0:w],
                            q2[:, h, qb : qb + 128],
                            k2[:, lo : lo + w],
                            start=False,
                            stop=True,
                        )
                        schunks.append(sp)
                    # masks: window-tail triangle on k-block qc-8, causal on qc
                    if kbs[0] == qc - 8:
                        nc.vector.tensor_add(
                            schunks[0][:, 0:128], schunks[0][:, 0:128], t0[:]
                        )
                    dpos = (nkb - 1) % 3
                    nc.vector.tensor_add(
                        schunks[-1][:, dpos * 128 : (dpos + 1) * 128],
                        schunks[-1][:, dpos * 128 : (dpos + 1) * 128],
                        t8[:],
                    )
                    # exp + row sums
                    p_sb = pp.tile([128, 1152], BF16, tag="p")
                    acc = smp.tile([128, 3], F32, tag="acc")
                    for ci, chunk in enumerate(chunks):
                        w = len(chunk) * 128
                        nc.scalar.activation(
                            p_sb[:, ci * 384 : ci * 384 + w],
                            schunks[ci][:, 0:w],
                            EXP,
                            accum_out=acc[:, ci : ci + 1],
                        )
                    sm = smp.tile([128, 1], F32, tag="sm")
                    if len(chunks) == 1:
                        nc.vector.tensor_copy(sm[:], acc[:, 0:1])
                    else:
                        nc.vector.tensor_add(sm[:], acc[:, 0:1], acc[:, 1:2])
                        if len(chunks) == 3:
                            nc.vector.tensor_add(sm[:], sm[:], acc[:, 2:3])
                    rc = smp.tile([128, 1], F32, tag="rc")
                    nc.vector.reciprocal(rc[:], sm[:])
                    dg = dgp.tile([128, 128], BF16, tag="dg")
                    nc.vector.tensor_scalar_mul(dg[:], ident[:], rc[:, 0:1])
                    # normalized transpose: PT[k,q] = P^T @ diag(1/sum)
                    pt_sb = ptp.tile([128, 1152], BF16, tag="pt")
                    for ci, chunk in enumerate(chunks):
                        w = len(chunk) * 128
                        ptps = ps.tile([128, 512], F32, tag="ps")
                        for t in range(len(chunk)):
                            nc.tensor.matmul(
                                ptps[:, t * 128 : (t + 1) * 128],
                                p_sb[:, ci * 384 + t * 128 : ci * 384 + (t + 1) * 128],
                                dg[:],
                                start=True,
                                stop=True,
                            )
                        nc.any.tensor_copy(
                            pt_sb[:, ci * 384 : ci * 384 + w], ptps[:, 0:w]
                        )
                    # PV
                    for mi, kb in enumerate(kbs):
                        ci, t = mi // 3, mi % 3
                        nc.tensor.matmul(
                            aps[:, qc4 * 128 : (qc4 + 1) * 128],
                            v_sb[:, kb, :],
                            pt_sb[:, ci * 384 + t * 128 : ci * 384 + (t + 1) * 128],
                            start=(mi == 0),
                            stop=(mi == nkb - 1),
                        )
                nc.any.tensor_copy(attn[:, h, qg * 512 : (qg + 1) * 512], aps[:])

            # partial O-projection for this q-group (overlaps next group's attn)
            for qc in range(qg * 4, (qg + 1) * 4):
                for dn in range(4):
                    ops = ps.tile([128, 512], F32, tag="ps")
                    for f in range(HPC):
                        nc.tensor.matmul(
                            ops[:],
                            attn[:, f, qc * 128 : (qc + 1) * 128],
                            wo_t[f // 2][:, f % 2, dn * 512 : (dn + 1) * 512],
                            start=(f == 0),
                            stop=(f == HPC - 1),
                        )
                    o_sb = op_.tile([128, 512], F32, tag="o")
                    nc.any.tensor_copy(o_sb[:], ops[:])
                    nc.gpsimd.dma_start(
                        out_d[qc * 128 : (qc + 1) * 128, dn * 512 : (dn + 1) * 512],
                        o_sb[:],
                    )

    nc.compile()
    return nc


def _prep_core(inputs, c):
    x = inputs["x"]
    cos, sin = np.asarray(inputs["cos"]), np.asarray(inputs["sin"])
    mask = np.asarray(inputs["mask"])
    wq = np.asarray(inputs["wq"], dtype=np.float32)
    wk = np.asarray(inputs["wk"], dtype=np.float32)
    wv = np.asarray(inputs["wv"], dtype=np.float32)
    wo = np.asarray(inputs["wo"], dtype=np.float32)
    bf = ml_dtypes.bfloat16
    b, g = c // 4, c % 4

    # x[b] transposed -> [128p, cq, dc, 512]
    xt = np.asarray(x[b], dtype=np.float32).T  # [dim, S]
    xt = xt.reshape(NDC, 128, 4, 512).transpose(1, 2, 0, 3)
    xt = np.ascontiguousarray(xt).reshape(128, 4 * NDC * 512).astype(bf)

    # wq slice for heads 4g..4g+3 (SCALE folded), [p, hpair, dc, 256]
    wqs = (wq[:, g * 512 : (g + 1) * 512] * SCALE).reshape(NDC, 128, 2, 256)
    wqs = np.ascontiguousarray(wqs.transpose(1, 2, 0, 3)).reshape(128, 2 * NDC * 256)
    # wk|wv slice for kv head g: [p, dc, 256] with cols [wk 128 | wv 128]
    wkv = np.concatenate(
        [wk[:, g * 128 : (g + 1) * 128], wv[:, g * 128 : (g + 1) * 128]], axis=1
    )
    wkv = np.ascontiguousarray(wkv.reshape(NDC, 128, 256).transpose(1, 0, 2)).reshape(
        128, NDC * 256
    )
    # wo rows for this core's heads: [p, f2(2 within pair), ...] tiles [128,2,2048]
    wos = wo[g * 512 : (g + 1) * 512].reshape(2, 2, 128, 2048).transpose(2, 0, 1, 3)
    wos = np.ascontiguousarray(wos).reshape(128, 2 * 2 * 2048)

    fm = np.ascontiguousarray((cos - sin).T, dtype=np.float32)
    fp_ = np.ascontiguousarray((cos + sin).T, dtype=np.float32)
    t0 = np.ascontiguousarray(mask[WINDOW : WINDOW + 128, 0:128], dtype=np.float32)
    t8 = np.ascontiguousarray(mask[0:128, 0:128], dtype=np.float32)

    return {
        "xt": xt, "wq": wqs.astype(bf), "wkv": wkv.astype(bf), "wo": wos.astype(bf),
        "fm": fm, "fp": fp_, "t0": t0, "t8": t8,
        "ident": np.eye(128, dtype=np.float32).astype(bf),
    }


def kernel(**inputs) -> np.ndarray:
    if "nc" not in _cache:
        _cache["nc"] = _build()
    nc = _cache["nc"]
    in_maps = [_prep_core(inputs, c) for c in range(8)]
    res = run_bass_kernel_spmd(nc, in_maps, core_ids=list(range(8)))
    out = np.zeros((B, S, DIM), dtype=np.float32)
    for c in range(8):
        out[c // 4] += res.results[c]["out"]
    return out


# revision 14
# speedup vs baseline: 1.2846x; 1.0993x over previous
"""Distributed Bass kernel for sliding-window GQA attention on 8 TRN2 NeuronCores.

Problem: B=2, S=2048, DIM=2048, H=16, KVH=4, HD=128, WINDOW=1024 (causal
sliding window), nonstandard RoPE producing 1.5*HD score features.

Sharding (tensor-parallel on the kv-head axis, data-parallel on batch —
no collectives): core c owns (batch, kv-group) = (c//4, c%4): its 4 q-heads
and 1 kv head over the full 2048-row sequence. wq/wk/wv are column-sharded
by kv group, wo row-sharded. Each core emits a PARTIAL output projection
(its 4 heads x its wo rows); the host sums the 4 partials per batch while
unsharding — replacing the all-reduce.

Per core: Q/K/V projections + rope scaling, block-sparse sliding-window
attention in global coordinates (k-blocks max(0,qc-8)..qc per 128-row
q-block qc), unnormalized exp softmax (bounded scores, no max pass), a
transpose-by-matmul against diag(1/rowsum) that normalizes for free, PV,
and the partial O-projection.
"""
import numpy as np
import ml_dtypes

import concourse.tile as tile
from concourse import bacc, mybir
from concourse.bass_utils import run_bass_kernel_spmd
from contextlib import ExitStack

F32 = mybir.dt.float32
BF16 = mybir.dt.bfloat16
EXP = mybir.ActivationFunctionType.Exp

B, S, DIM = 2, 2048, 2048
H, KVH, HD = 16, 4, 128
HPC = H // KVH  # heads per core (4)
WINDOW = 1024
SCALE = HD ** -0.5
NDC = DIM // 128  # 16 dim chunks
NQC = S // 128    # 16 q blocks

_cache = {}


def _kblocks(qc):
    return list(range(max(0, qc - 8), qc + 1))


def _build():
    nc = bacc.Bacc("TRN2", target_bir_lowering=False, debug=False, num_devices=8)

    xt_d = nc.dram_tensor("xt", [128, 4 * NDC * 512], BF16, kind="ExternalInput")
    wq_d = nc.dram_tensor("wq", [128, 2 * NDC * 256], BF16, kind="ExternalInput")
    wkv_d = nc.dram_tensor("wkv", [128, NDC * 256], BF16, kind="ExternalInput")
    wo_d = nc.dram_tensor("wo", [128, 2 * 2 * 2048], BF16, kind="ExternalInput")
    fm_d = nc.dram_tensor("fm", [64, S], F32, kind="ExternalInput")
    fp_d = nc.dram_tensor("fp", [64, S], F32, kind="ExternalInput")
    t0_d = nc.dram_tensor("t0", [128, 128], F32, kind="ExternalInput")
    t8_d = nc.dram_tensor("t8", [128, 128], F32, kind="ExternalInput")
    id_d = nc.dram_tensor("ident", [128, 128], BF16, kind="ExternalInput")
    out_d = nc.dram_tensor("out", [S, DIM], F32, kind="ExternalOutput")

    with tile.TileContext(nc) as tc, ExitStack() as ctx:
        xp = ctx.enter_context(tc.tile_pool(name="xp", bufs=3))
        wp = ctx.enter_context(tc.tile_pool(name="wp", bufs=3))
        cp = ctx.enter_context(tc.tile_pool(name="cp", bufs=1))
        qp = ctx.enter_context(tc.tile_pool(name="qp", bufs=1))
        kp = ctx.enter_context(tc.tile_pool(name="kp", bufs=1))
        vp = ctx.enter_context(tc.tile_pool(name="vp", bufs=1))
        pp = ctx.enter_context(tc.tile_pool(name="pp", bufs=2))
        ptp = ctx.enter_context(tc.tile_pool(name="ptp", bufs=2))
        dgp = ctx.enter_context(tc.tile_pool(name="dgp", bufs=2))
        smp = ctx.enter_context(tc.tile_pool(name="smp", bufs=8))
        ap_ = ctx.enter_context(tc.tile_pool(name="ap", bufs=1))
        op_ = ctx.enter_context(tc.tile_pool(name="op", bufs=3))
        ps = ctx.enter_context(tc.tile_pool(name="ps", bufs=5, space="PSUM"))
        sps = ctx.enter_context(tc.tile_pool(name="sps", bufs=3, space="PSUM"))

        # ---- weights for phase 1 first (prologue-critical DMA order) ----
        wkv_t = wp.tile([128, NDC, 256], BF16, tag="w")  # cols: [wk 128 | wv 128]
        nc.sync.dma_start(wkv_t[:], wkv_d[:, :])
        wq_t = None  # allocated after the first x chunk's DMAs

        q1 = qp.tile([128, HPC, S], BF16, tag="q1")
        q2 = qp.tile([64, HPC, S], BF16, tag="q2")
        k1 = kp.tile([128, S], BF16, tag="k1")
        k2 = kp.tile([64, S], BF16, tag="k2")
        v_sb = vp.tile([128, NQC, 128], BF16, tag="v")

        # ---- projections, streamed per x column-quarter ----
        fm = fp = t0 = t8 = ident = None
        for cq in range(4):
            x_q = xp.tile([128, NDC, 512], BF16, tag="x")
            for dg in range(4):
                nc.sync.dma_start(
                    x_q[:, dg * 4 : (dg + 1) * 4, :],
                    xt_d[
                        :,
                        cq * NDC * 512 + dg * 4 * 512 : cq * NDC * 512
                        + (dg + 1) * 4 * 512,
                    ],
                )
            if cq == 0:
                # wq + constants ride after the first x chunk (not prologue-critical)
                wq_t = [
                    wp.tile([128, NDC, 256], BF16, tag="w", name=f"wq{i}")
                    for i in range(2)
                ]
                for i in range(2):
                    nc.sync.dma_start(
                        wq_t[i][:], wq_d[:, i * NDC * 256 : (i + 1) * NDC * 256]
                    )
                fm = cp.tile([64, S], F32, tag="fm")
                nc.sync.dma_start(fm[:], fm_d[:, :])
                fp = cp.tile([64, S], F32, tag="fp")
                nc.sync.dma_start(fp[:], fp_d[:, :])
                t0 = cp.tile([128, 128], F32, tag="t0")
                nc.sync.dma_start(t0[:], t0_d[:, :])
                t8 = cp.tile([128, 128], F32, tag="t8")
                nc.sync.dma_start(t8[:], t8_d[:, :])
                ident = cp.tile([128, 128], BF16, tag="ident")
                nc.sync.dma_start(ident[:], id_d[:, :])
            cs = slice(cq * 512, (cq + 1) * 512)
            fmc, fpc = fm[:, cs], fp[:, cs]

            kps = ps.tile([128, 512], F32, tag="ps")
            for dc in range(NDC):
                nc.tensor.matmul(
                    kps[:],
                    wkv_t[:, dc, 0:128],
                    x_q[:, dc, :],
                    start=(dc == 0),
                    stop=(dc == NDC - 1),
                )
            nc.vector.tensor_mul(k1[0:64, cs], kps[0:64, :], fmc)
            nc.vector.tensor_mul(k1[64:128, cs], kps[0:64, :], fpc)
            nc.scalar.copy(k2[:, cs], kps[64:128, :])

            vps = ps.tile([128, 512], F32, tag="ps")
            for kb4 in range(4):
                kb = cq * 4 + kb4
                for dc in range(NDC):
                    nc.tensor.matmul(
                        vps[:, kb4 * 128 : (kb4 + 1) * 128],
                        x_q[:, dc, kb4 * 128 : (kb4 + 1) * 128],
                        wkv_t[:, dc, 128:256],
                        start=(dc == 0),
                        stop=(dc == NDC - 1),
                    )
            for kb4 in range(4):
                nc.any.tensor_copy(
                    v_sb[:, cq * 4 + kb4, :], vps[:, kb4 * 128 : (kb4 + 1) * 128]
                )

            for h in range(HPC):
                qps = ps.tile([128, 512], F32, tag="ps")
                for dc in range(NDC):
                    nc.tensor.matmul(
                        qps[:],
                        wq_t[h // 2][:, dc, (h % 2) * 128 : (h % 2 + 1) * 128],
                        x_q[:, dc, :],
                        start=(dc == 0),
                        stop=(dc == NDC - 1),
                    )
                nc.vector.tensor_mul(q1[0:64, h, cs], qps[0:64, :], fmc)
                nc.vector.tensor_mul(q1[64:128, h, cs], qps[0:64, :], fpc)
                nc.scalar.copy(q2[:, h, cs], qps[64:128, :])

        # ---- attention + interleaved partial O-projection per q-group ----
        wo_t = [
            wp.tile([128, 2, 2048], BF16, tag="w", name=f"wo{i}") for i in range(2)
        ]
        for i in range(2):
            nc.sync.dma_start(wo_t[i][:], wo_d[:, i * 4096 : (i + 1) * 4096])
        attn = ap_.tile([128, HPC, S], BF16, tag="attn")
        for qg in range(4):  # q groups of 512 (4 q-blocks) per attn psum
            for h in range(HPC):
                aps = ps.tile([128, 512], F32, tag="ps")
                for qc4 in range(4):
                    qc = qg * 4 + qc4
                    qb = qc * 128
                    kbs = _kblocks(qc)
                    nkb = len(kbs)
                    chunks = [kbs[i : i + 3] for i in range(0, nkb, 3)]
                    schunks = []
                    for chunk in chunks:
                        w = len(chunk) * 128
                        sp = sps.tile([128, 384], F32, tag="s")
                        lo = chunk[0] * 128
                        nc.tensor.matmul(
                            sp[:, 0:w],
                            q1[:, h, qb : qb + 128],
                            k1[:, lo : lo + w],
                            start=True,
                            stop=False,
                        )
                        nc.tensor.matmul(
                            sp[:, 0:w],
                            q2[:, h, qb : qb + 128],
                            k2[:, lo : lo + w],
                            start=False,
                            stop=True,
                        )
                        schunks.append(sp)
                    # masks: window-tail triangle on k-block qc-8, causal on qc
                    if kbs[0] == qc - 8:
                        nc.vector.tensor_add(
                            schunks[0][:, 0:128], schunks[0][:, 0:128], t0[:]
                        )
                    dpos = (nkb - 1) % 3
                    nc.vector.tensor_add(
                        schunks[-1][:, dpos * 128 : (dpos + 1) * 128],
                        schunks[-1][:, dpos * 128 : (dpos + 1) * 128],
                        t8[:],
                    )
                    # exp + row sums
                    p_sb = pp.tile([128, 1152], BF16, tag="p")
                    acc = smp.tile([128, 3], F32, tag="acc")
                    for ci, chunk in enumerate(chunks):
                        w = len(chunk) * 128
                        nc.scalar.activation(
                            p_sb[:, ci * 384 : ci * 384 + w],
                            schunks[ci][:, 0:w],
                            EXP,
                            accum_out=acc[:, ci : ci + 1],
                        )
                    sm = smp.tile([128, 1], F32, tag="sm")
                    if len(chunks) == 1:
                        nc.vector.tensor_copy(sm[:], acc[:, 0:1])
                    else:
                        nc.vector.tensor_add(sm[:], acc[:, 0:1], acc[:, 1:2])
                        if len(chunks) == 3:
                            nc.vector.tensor_add(sm[:], sm[:], acc[:, 2:3])
                    rc = smp.tile([128, 1], F32, tag="rc")
                    nc.vector.reciprocal(rc[:], sm[:])
                    dg = dgp.tile([128, 128], BF16, tag="dg")
                    nc.vector.tensor_scalar_mul(dg[:], ident[:], rc[:, 0:1])
                    # normalized transpose: PT[k,q] = P^T @ diag(1/sum)
                    pt_sb = ptp.tile([128, 1152], BF16, tag="pt")
                    for ci, chunk in enumerate(chunks):
                        w = len(chunk) * 128
                        ptps = ps.tile([128, 512], F32, tag="ps")
                        for t in range(len(chunk)):
                            nc.tensor.matmul(
                                ptps[:, t * 128 : (t + 1) * 128],
                                p_sb[:, ci * 384 + t * 128 : ci * 384 + (t + 1) * 128],
                                dg[:],
                                start=True,
                                stop=True,
                            )
                        nc.any.tensor_copy(
                            pt_sb[:, ci * 384 : ci * 384 + w], ptps[:, 0:w]
                        )
                    # PV
                    for mi, kb in enumerate(kbs):
                        ci, t = mi // 3, mi % 3
                        nc.tensor.matmul(
                            aps[:, qc4 * 128 : (qc4 + 1) * 128],
                            v_sb[:, kb, :],
                            pt_sb[:, ci * 384 + t * 128 : ci * 384 + (t + 1) * 128],
                            start=(mi == 0),
                            stop=(mi == nkb - 1),
                        )
                nc.any.tensor_copy(attn[:, h, qg * 512 : (qg + 1) * 512], aps[:])

            # partial O-projection for this q-group (overlaps next group's attn)
            for qc in range(qg * 4, (qg + 1) * 4):
                o_sb = op_.tile([128, 2048], F32, tag="o")
                for dn in range(4):
                    ops = ps.tile([128, 512], F32, tag="ps")
                    for f in range(HPC):
                        nc.tensor.matmul(
                            ops[:],
                            attn[:, f, qc * 128 : (qc + 1) * 128],
                            wo_t[f // 2][:, f % 2, dn * 512 : (dn + 1) * 512],
                            start=(f == 0),
                            stop=(f == HPC - 1),
                        )
                    nc.any.tensor_copy(o_sb[:, dn * 512 : (dn + 1) * 512], ops[:])
                nc.sync.dma_start(out_d[qc * 128 : (qc + 1) * 128, :], o_sb[:])

    nc.compile()
    return nc


def _prep_core(inputs, c):
    x = inputs["x"]
    cos, sin = np.asarray(inputs["cos"]), np.asarray(inputs["sin"])
    mask = np.asarray(inputs["mask"])
    wq = np.asarray(inputs["wq"], dtype=np.float32)
    wk = np.asarray(inputs["wk"], dtype=np.float32)
    wv = np.asarray(inputs["wv"], dtype=np.float32)
    wo = np.asarray(inputs["wo"], dtype=np.float32)
    bf = ml_dtypes.bfloat16
    b, g = c // 4, c % 4

    # x[b] transposed -> [128p, cq, dc, 512]
    xt = np.asarray(x[b], dtype=np.float32).T  # [dim, S]
    xt = xt.reshape(NDC, 128, 4, 512).transpose(1, 2, 0, 3)
    xt = np.ascontiguousarray(xt).reshape(128, 4 * NDC * 512).astype(bf)

    # wq slice for heads 4g..4g+3 (SCALE folded), [p, hpair, dc, 256]
    wqs = (wq[:, g * 512 : (g + 1) * 512] * SCALE).reshape(NDC, 128, 2, 256)
    wqs = np.ascontiguousarray(wqs.transpose(1, 2, 0, 3)).reshape(128, 2 * NDC * 256)
    # wk|wv slice for kv head g: [p, dc, 256] with cols [wk 128 | wv 128]
    wkv = np.concatenate(
        [wk[:, g * 128 : (g + 1) * 128], wv[:, g * 128 : (g + 1) * 128]], axis=1
    )
    wkv = np.ascontiguousarray(wkv.reshape(NDC, 128, 256).transpose(1, 0, 2)).reshape(
        128, NDC * 256
    )
    # wo rows for this core's heads: [p, f2(2 within pair), ...] tiles [128,2,2048]
    wos = wo[g * 512 : (g + 1) * 512].reshape(2, 2, 128, 2048).transpose(2, 0, 1, 3)
    wos = np.ascontiguousarray(wos).reshape(128, 2 * 2 * 2048)

    fm = np.ascontiguousarray((cos - sin).T, dtype=np.float32)
    fp_ = np.ascontiguousarray((cos + sin).T, dtype=np.float32)
    t0 = np.ascontiguousarray(mask[WINDOW : WINDOW + 128, 0:128], dtype=np.float32)
    t8 = np.ascontiguousarray(mask[0:128, 0:128], dtype=np.float32)

    return {
        "xt": xt, "wq": wqs.astype(bf), "wkv": wkv.astype(bf), "wo": wos.astype(bf),
        "fm": fm, "fp": fp_, "t0": t0, "t8": t8,
        "ident": np.eye(128, dtype=np.float32).astype(bf),
    }


def kernel(**inputs) -> np.ndarray:
    if "nc" not in _cache:
        _cache["nc"] = _build()
    nc = _cache["nc"]
    in_maps = [_prep_core(inputs, c) for c in range(8)]
    res = run_bass_kernel_spmd(nc, in_maps, core_ids=list(range(8)))
    out = np.zeros((B, S, DIM), dtype=np.float32)
    for c in range(8):
        out[c // 4] += res.results[c]["out"]
    return out
